# revision 32
# baseline (speedup 1.0000x reference)
"""Trainium2 Bass kernel for nn_ConfigurationLXMERTDecoder (retrieval_knn).

Data-parallel across 8 NeuronCores: batch 64 -> 8 per core. All compute on
device; host only reshapes/shards inputs into the SBUF-friendly layouts the
kernel DMAs in, and concatenates the per-core outputs.

Outputs (matching reference): (h_1, c_1, logit, h_tilde, ctx_attn)
"""

import os
import sys

sys.path.insert(0, "/opt/trn_rl_repo")

from contextlib import ExitStack

import numpy as np

import concourse.bass as bass
import concourse.bacc as bacc
import concourse.tile as tile
from concourse import mybir
from concourse.bass_utils import run_bass_kernel_spmd

# model dims
B, S, L, C, OBJ = 64, 80, 8, 12, 36
AFS, E, H, FEAT = 4, 64, 512, 2052
TOPN = 3
OBJ_D = 2048
SL = S * L  # 640

NCORES = 8
BL = B // NCORES  # 8 batch per core
FEATP = 2176  # FEAT padded to 17*128
KF = FEATP // 128  # 17
KH = H // 128  # 4
BS_F = BL * OBJ  # 288  (b,s) pairs for feature attention
BS_C = BL * S  # 640  (b,s) pairs for ctx attention
PC = BL * C  # 96   (b,c) pairs
NROW = PC * OBJ  # 3456 obj rows per core
KCAND = 8704  # 8196 padded to 17*512
NIDX = 384  # gather slots (3 cols x 128 partitions)
NT_IH = 512  # gates col-tile
NT_K = 512  # tgt_k col-tile

F32 = mybir.dt.float32
F32R = mybir.dt.float32r
BF16 = mybir.dt.float16  # 16-bit matmul dtype (fp16: 10-bit mantissa)
I16 = mybir.dt.int16
U32 = mybir.dt.uint32

USE_F32R = False  # stream fp32 matmul operands as float32r (full-rate) via bitcast
KPHASE = int(os.environ.get("KPHASE", "9"))  # debug: stop after phase N
KSTEP = int(os.environ.get("KSTEP", "9"))  # debug: stop mid-phase-E

_COMPILED = None


def _ap(t, ap_dims, offset=0):
    base = t[:]
    return bass.AP(tensor=base.tensor, offset=base.offset + offset, ap=ap_dims)


def _dram_ap(d, ap_dims, offset=0):
    return bass.AP(tensor=d.ap().tensor, offset=offset, ap=ap_dims)


def _mm(nc, out, lhsT, rhs, start, stop):
    if USE_F32R:
        lhsT = lhsT.bitcast(F32R)
        rhs = rhs.bitcast(F32R)
    nc.tensor.matmul(out=out, lhsT=lhsT, rhs=rhs, start=start, stop=stop)


def _softmax(nc, pool, out, logits, n):
    """out = softmax(logits) along free dim (logits bounded: no max-subtract)."""
    p = logits.shape[0]
    ex = pool.tile([p, n], F32, tag="sm_exp", name="ex")
    ssum = pool.tile([p, 1], F32, tag="sm_sum", name="ssum")
    nc.scalar.activation(
        out=ex[:], in_=logits[:], func=mybir.ActivationFunctionType.Exp,
        accum_out=ssum[:],
    )
    rsum = pool.tile([p, 1], F32, tag="sm_rsum", name="rsum")
    nc.vector.reciprocal(out=rsum[:], in_=ssum[:])
    nc.vector.tensor_tensor(
        out=out[:], in0=ex[:], in1=rsum.to_broadcast([p, n]), op=mybir.AluOpType.mult,
    )


def build_kernel(tc, ins, outs):
    nc = tc.nc
    ctx = ExitStack()
    with ctx:
        persist = ctx.enter_context(tc.tile_pool(name="persist", bufs=1))
        small = ctx.enter_context(tc.tile_pool(name="small", bufs=1))
        psum = ctx.enter_context(tc.tile_pool(name="psum", bufs=1, space="PSUM"))
        psum_s = ctx.enter_context(tc.tile_pool(name="psum_s", bufs=2, space="PSUM"))
        psum_bc = ctx.enter_context(tc.tile_pool(name="psum_bc", bufs=2,
                                                 space="PSUM"))
        # weight streaming pools (kernel-lifetime so DMA prefetch is unblocked)
        poolWih = ctx.enter_context(tc.tile_pool(name="wih", bufs=2))
        poolWhh = ctx.enter_context(tc.tile_pool(name="whh", bufs=2))
        poolWk = ctx.enter_context(tc.tile_pool(name="wk", bufs=4))
        poolWf = ctx.enter_context(tc.tile_pool(name="wf", bufs=2))

        def big_tile(shape):
            return psum.tile(shape, F32, tag="big", name="ps_big")

        def sm_tile(shape):
            return psum_s.tile(shape, F32, tag="sm", name="ps_sm")

        # ---- persistent small loads
        sb_eye8 = persist.tile([8, 8], F32)
        nc.scalar.dma_start(out=sb_eye8[:], in_=ins["eye8"])
        sb_actionT5 = persist.tile([5, BL], F32)
        nc.scalar.dma_start(out=sb_actionT5[:], in_=ins["actionT5"])
        sb_WembT5 = persist.tile([5, E], F32)
        nc.scalar.dma_start(out=sb_WembT5[:], in_=ins["WembT5"])
        sb_h1T = persist.tile([128, KH, BL], BF16)
        nc.scalar.dma_start(out=sb_h1T[:], in_=ins["h1T"])
        sb_c0 = persist.tile([BL, H], F32)
        nc.scalar.dma_start(out=sb_c0[:], in_=ins["c0"])
        sb_maskadd = persist.tile([BL, S], F32)
        nc.scalar.dma_start(out=sb_maskadd[:], in_=ins["maskadd"])
        sb_landmark = persist.tile([BL, SL], F32)
        nc.scalar.dma_start(out=sb_landmark[:], in_=ins["landmark"])
        sb_iota = persist.tile([PC, SL], F32)
        nc.scalar.dma_start(out=sb_iota[:], in_=ins["iota640"])
        sb_sim1f = persist.tile([PC, SL], F32)
        nc.scalar.dma_start(out=sb_sim1f[:], in_=ins["sim1f"])
        sb_rowbase = persist.tile([PC, 1], F32)
        nc.scalar.dma_start(out=sb_rowbase[:], in_=ins["rowbase"])
        sb_candf = persist.tile([PC, FEAT], F32)
        nc.scalar.dma_start(out=sb_candf[:], in_=ins["candf"])
        sb_Wih_emb = persist.tile([E + 1, 4 * H], BF16)
        nc.scalar.dma_start(out=sb_Wih_emb[:], in_=ins["Wih_emb65"])

        sb_afT = persist.tile([128, KF, BL], BF16)  # attn_feat^T chunks (lhsT)
        sb_oh12 = persist.tile([BL, PC], BF16)  # onehot12[b, 12b+c] = 1
        nc.gpsimd.dma_start(out=sb_oh12[:], in_=ins["onehot12"])
        sb_ones128 = persist.tile([1, 128], BF16)
        nc.gpsimd.dma_start(out=sb_ones128[:], in_=ins["ones128"])
        sb_oh12f = persist.tile([BL, PC], F32)
        nc.gpsimd.dma_start(out=sb_oh12f[:], in_=ins["onehot12f"])

        # ---------------- Phase A: action embedding  embT [E, BL]
        ps_emb = sm_tile([E, BL])
        _mm(nc, ps_emb[:], sb_WembT5[:], sb_actionT5[:], True, True)
        sb_embT65 = persist.tile([E + 1, BL], BF16)
        nc.scalar.activation(
            out=sb_embT65[0:E, :], in_=ps_emb[:],
            func=mybir.ActivationFunctionType.Tanh,
        )
        nc.vector.memset(sb_embT65[E : E + 1, :], 1.0)
        if KPHASE <= 1:
            return

        # ---------------- Phase B: feature attention
        with tc.tile_pool(name="phaseB", bufs=1) as poolB:
            sb_featT = poolB.tile([128, KF, BS_F], BF16)
            nc.sync.dma_start(out=sb_featT[:], in_=ins["featT"])

            # tgt_f = prev_h1 @ W_in_feat.T ; 5 col-tiles (4x512 + 128)
            sb_tgtf = poolB.tile([BL, FEATP], F32)
            for q in range(5):
                qn = 512 if q < 4 else 128
                wq = poolWf.tile([128, KH, 512], BF16, tag="wf", name="wf_q")
                nc.sync.dma_start(
                    out=wq[:, :, 0:qn],
                    in_=ins["WfT"][:, :, 512 * q : 512 * q + qn],
                )
                ps_q = sm_tile([BL, 512])
                for k in range(KH):
                    _mm(
                        nc, ps_q[:, 0:qn], sb_h1T[:, k, :],
                        wq[:, k, 0:qn], k == 0, k == KH - 1,
                    )
                nc.scalar.activation(
                    out=sb_tgtf[:, 512 * q : 512 * q + qn], in_=ps_q[:, 0:qn],
                    func=mybir.ActivationFunctionType.Copy,
                )

            # transpose -> tgt_f^T [128, KF, BL]
            ps_tfT = sm_tile([128, KF, BL])
            for t in range(KF):
                nc.tensor.transpose(
                    ps_tfT[:, t, :], sb_tgtf[:, 128 * t : 128 * (t + 1)], sb_eye8[:]
                )
            sb_tfT = poolB.tile([128, KF, BL], BF16)
            nc.vector.tensor_copy(out=sb_tfT[:], in_=ps_tfT[:])

            # logits: lf[b, (b',s)] block-diag useful  -> psum [BL, BS_F]
            ps_lf = sm_tile([BL, BS_F])
            for k in range(KF):
                _mm(nc, ps_lf[:], sb_tfT[:, k, :], sb_featT[:, k, :], k == 0, k == KF - 1)
            sb_lfull = small.tile([BL, BS_F], F32, tag="lfull")
            nc.scalar.activation(out=sb_lfull[:], in_=ps_lf[:],
                                 func=mybir.ActivationFunctionType.Copy)
            sb_lf = small.tile([BL, OBJ], F32, tag="lf")
            pit = sb_lfull[:].ap[0][0]
            nc.gpsimd.dma_start(
                out=sb_lf[:], in_=_ap(sb_lfull, [[pit + OBJ, BL], [1, OBJ]])
            )
            sb_attnf = small.tile([BL, OBJ], F32, tag="attnf")
            _softmax(nc, small, sb_attnf, sb_lf, OBJ)

            # attn_feat^T = sum_s attn[b,s] * featT[:, :, (b,s)]
            # broadcast attn to all 128 partitions via PE: ones^T @ attn_flat
            sb_attnf16 = small.tile([BL, OBJ], BF16, tag="attnf16")
            nc.vector.tensor_copy(out=sb_attnf16[:], in_=sb_attnf[:])
            sb_aflat = small.tile([1, BS_F], BF16, tag="aflat")
            nc.gpsimd.dma_start(out=sb_aflat[:], in_=sb_attnf16[:])
            ps_ab = psum_bc.tile([128, BS_F], F32, tag="bc", name="ps_ab")
            _mm(nc, ps_ab[:], sb_ones128[:], sb_aflat[:], True, True)
            nc.vector.tensor_tensor(
                out=sb_featT[:], in0=sb_featT[:],
                in1=_ap(ps_ab, [[BS_F, 128], [0, KF], [1, BS_F]]),
                op=mybir.AluOpType.mult,
            )
            sb_afTf = poolB.tile([128, KF, BL], F32)
            nc.vector.tensor_reduce(
                out=sb_afTf[:],
                in_=sb_featT[:].rearrange("p k (b s) -> p k b s", b=BL),
                axis=mybir.AxisListType.X, op=mybir.AluOpType.add,
            )
            nc.vector.tensor_copy(out=sb_afT[:], in_=sb_afTf[:])

        if KPHASE <= 2:
            return

        # ---------------- Phase C: LSTM gates + cell
        ps_gates = big_tile([BL, 4 * H])
        for n in range(4 * H // NT_IH):  # 4 tiles of 512
            cols = slice(NT_IH * n, NT_IH * (n + 1))
            _mm(nc, ps_gates[:, cols], sb_embT65[:], sb_Wih_emb[:, cols], True, False)
            wtf = poolWih.tile([128, KF, NT_IH], BF16, tag="wihf", name="wtf")
            nc.sync.dma_start(out=wtf[:], in_=ins["Wih_feat"][n])
            for k in range(KF):
                _mm(nc, ps_gates[:, cols], sb_afT[:, k, :], wtf[:, k, :], False, False)
            wth = poolWhh.tile([128, KH, NT_IH], BF16, tag="whh", name="wth")
            nc.sync.dma_start(out=wth[:], in_=ins["Whh"][n])
            for k in range(KH):
                _mm(
                    nc, ps_gates[:, cols], sb_h1T[:, k, :], wth[:, k, :],
                    False, k == KH - 1,
                )

        # gates reordered on host to [i, f, o, g]
        sb_sig = small.tile([BL, 3 * H], F32, tag="sig")
        nc.scalar.activation(out=sb_sig[:], in_=ps_gates[:, 0 : 3 * H],
                             func=mybir.ActivationFunctionType.Sigmoid)
        sb_gg = small.tile([BL, H], F32, tag="gg")
        nc.scalar.activation(out=sb_gg[:], in_=ps_gates[:, 3 * H : 4 * H],
                             func=mybir.ActivationFunctionType.Tanh)
        sb_c1 = persist.tile([BL, H], F32)
        sb_t1 = small.tile([BL, H], F32, tag="t1")
        nc.vector.tensor_tensor(out=sb_t1[:], in0=sb_sig[:, H : 2 * H],
                                in1=sb_c0[:], op=mybir.AluOpType.mult)
        nc.vector.tensor_tensor(out=sb_gg[:], in0=sb_sig[:, 0:H], in1=sb_gg[:],
                                op=mybir.AluOpType.mult)
        nc.vector.tensor_tensor(out=sb_c1[:], in0=sb_t1[:], in1=sb_gg[:],
                                op=mybir.AluOpType.add)
        sb_tanhc1 = small.tile([BL, H], F32, tag="tanhc1")
        nc.scalar.activation(out=sb_tanhc1[:], in_=sb_c1[:],
                             func=mybir.ActivationFunctionType.Tanh)
        sb_h1 = persist.tile([BL, H], F32)
        nc.vector.tensor_tensor(out=sb_h1[:], in0=sb_sig[:, 2 * H : 3 * H],
                                in1=sb_tanhc1[:], op=mybir.AluOpType.mult)
        nc.gpsimd.dma_start(out=outs["out_c1"], in_=sb_c1[:])
        nc.gpsimd.dma_start(out=outs["out_h1"], in_=sb_h1[:])

        # h1^T for later matmuls
        ps_h1T = sm_tile([128, KH, BL])
        for t in range(KH):
            nc.tensor.transpose(
                ps_h1T[:, t, :], sb_h1[:, 128 * t : 128 * (t + 1)], sb_eye8[:]
            )
        sb_h1T4 = persist.tile([128, KH, BL], BF16)
        nc.vector.tensor_copy(out=sb_h1T4[:], in_=ps_h1T[:])

        if KPHASE <= 3:
            return

        # ---------------- Phase D: ctx attention
        sb_attnc = persist.tile([BL, S], F32)
        sb_ht = persist.tile([BL, H], F32)
        with tc.tile_pool(name="phaseD", bufs=1) as poolD:
            sb_ctxT = poolD.tile([128, KH, BS_C], BF16)
            nc.sync.dma_start(out=sb_ctxT[:], in_=ins["ctxT"])
            sb_WcT = poolD.tile([128, KH, H], BF16)
            nc.scalar.dma_start(out=sb_WcT[:], in_=ins["WcT"])
            sb_WoT = poolD.tile([128, 2 * KH, H], BF16)
            nc.scalar.dma_start(out=sb_WoT[:], in_=ins["WoT"])

            ps_tgtc = sm_tile([BL, H])
            for k in range(KH):
                _mm(nc, ps_tgtc[:], sb_h1T4[:, k, :], sb_WcT[:, k, :], k == 0, k == KH - 1)
            sb_tgtc = small.tile([BL, H], F32, tag="tgtc")
            nc.scalar.activation(out=sb_tgtc[:], in_=ps_tgtc[:],
                                 func=mybir.ActivationFunctionType.Copy)
            ps_tcT = sm_tile([128, KH, BL])
            for t in range(KH):
                nc.tensor.transpose(
                    ps_tcT[:, t, :], sb_tgtc[:, 128 * t : 128 * (t + 1)], sb_eye8[:]
                )
            sb_tcT = small.tile([128, KH, BL], BF16, tag="tcT")
            nc.vector.tensor_copy(out=sb_tcT[:], in_=ps_tcT[:])

            ps_lc = big_tile([BL, BS_C])
            for k in range(KH):
                _mm(nc, ps_lc[:, 0:512], sb_tcT[:, k, :], sb_ctxT[:, k, 0:512],
                    k == 0, k == KH - 1)
            for k in range(KH):
                _mm(nc, ps_lc[:, 512:640], sb_tcT[:, k, :], sb_ctxT[:, k, 512:640],
                    k == 0, k == KH - 1)
            sb_lcfull = small.tile([BL, BS_C], F32, tag="lcfull")
            nc.scalar.activation(out=sb_lcfull[:], in_=ps_lc[:],
                                 func=mybir.ActivationFunctionType.Copy)
            sb_lc = small.tile([BL, S], F32, tag="lc")
            pitc = sb_lcfull[:].ap[0][0]
            nc.gpsimd.dma_start(
                out=sb_lc[:], in_=_ap(sb_lcfull, [[pitc + S, BL], [1, S]])
            )
            nc.vector.tensor_tensor(out=sb_lc[:], in0=sb_lc[:], in1=sb_maskadd[:],
                                    op=mybir.AluOpType.add)
            _softmax(nc, small, sb_attnc, sb_lc, S)
            nc.gpsimd.dma_start(out=outs["out_attn"], in_=sb_attnc[:])

            # wctx^T via weighted sum over s; PE broadcast in 2 b-aligned tiles
            sb_attnc16 = small.tile([BL, S], BF16, tag="attnc16")
            nc.vector.tensor_copy(out=sb_attnc16[:], in_=sb_attnc[:])
            sb_acflat = small.tile([1, BS_C], BF16, tag="acflat")
            nc.gpsimd.dma_start(out=sb_acflat[:], in_=sb_attnc16[:])
            for lo, w in ((0, 400), (400, 240)):
                ps_ac = psum_bc.tile([128, 512], F32, tag="bc", name="ps_ac")
                _mm(nc, ps_ac[:, 0:w], sb_ones128[:], sb_acflat[:, lo : lo + w],
                    True, True)
                nc.vector.tensor_tensor(
                    out=sb_ctxT[:, :, lo : lo + w],
                    in0=sb_ctxT[:, :, lo : lo + w],
                    in1=_ap(ps_ac, [[512, 128], [0, KH], [1, w]]),
                    op=mybir.AluOpType.mult,
                )
            sb_wcTf = small.tile([128, KH, BL], F32, tag="wcTf")
            nc.vector.tensor_reduce(
                out=sb_wcTf[:],
                in_=sb_ctxT[:].rearrange("p k (b s) -> p k b s", b=BL),
                axis=mybir.AxisListType.X, op=mybir.AluOpType.add,
            )
            sb_wcT = small.tile([128, KH, BL], BF16, tag="wcT")
            nc.vector.tensor_copy(out=sb_wcT[:], in_=sb_wcTf[:])

            ps_ht = sm_tile([BL, H])
            for k in range(2 * KH):
                lhsT = sb_wcT[:, k, :] if k < KH else sb_h1T4[:, k - KH, :]
                _mm(nc, ps_ht[:], lhsT, sb_WoT[:, k, :], k == 0, k == 2 * KH - 1)
            nc.scalar.activation(out=sb_ht[:], in_=ps_ht[:],
                                 func=mybir.ActivationFunctionType.Tanh)
            nc.gpsimd.dma_start(out=outs["out_htilde"], in_=sb_ht[:])

        # h_tilde^T for tgt_k
        ps_htT = sm_tile([128, KH, BL])
        for t in range(KH):
            nc.tensor.transpose(
                ps_htT[:, t, :], sb_ht[:, 128 * t : 128 * (t + 1)], sb_eye8[:]
            )
        sb_htT = persist.tile([128, KH, BL], BF16)
        nc.vector.tensor_copy(out=sb_htT[:], in_=ps_htT[:])

        if KPHASE <= 4:
            return

        # ---------------- Phase E: top-3 + sim gather indices
        sb_scores = small.tile([BL, SL], F32, tag="scores")
        nc.vector.tensor_tensor(
            out=sb_scores[:], in0=sb_landmark[:],
            in1=_ap(sb_attnc, [[S, BL], [1, S], [0, L]]),
            op=mybir.AluOpType.mult,
        )
        sb_v8 = small.tile([BL, 8], F32, tag="v8")
        nc.vector.max(out=sb_v8[:], in_=sb_scores[:])
        sb_i8 = small.tile([BL, 8], U32, tag="i8")
        nc.vector.max_index(out=sb_i8[:], in_max=sb_v8[:], in_values=sb_scores[:])
        if KSTEP <= 1:
            return
        sb_i3f = small.tile([BL, TOPN], F32, tag="i3f")
        nc.vector.tensor_copy(out=sb_i3f[:], in_=sb_i8[:, 0:TOPN])
        if "dbg_i3" in outs:
            nc.sync.dma_start(out=outs["dbg_i3"], in_=sb_i3f[:])
        sb_onehot = small.tile([BL, TOPN, SL], F32, tag="onehot")
        nc.vector.tensor_tensor(
            out=sb_onehot[:],
            in0=_ap(sb_iota, [[SL, BL], [0, TOPN], [1, SL]]),
            in1=_ap(sb_i3f, [[TOPN, BL], [1, TOPN], [0, SL]]),
            op=mybir.AluOpType.is_equal,
        )
        if KSTEP <= 2:
            return
        dram_oh = nc.dram_tensor("dram_oh", [BL, TOPN * SL], F32)
        nc.sync.dma_start(out=dram_oh.ap(), in_=sb_onehot[:])
        sb_idxbuf = persist.tile([128, NIDX // 16], I16)
        with tc.tile_pool(name="phaseE", bufs=1) as poolE:
            sb_oh96 = poolE.tile([PC, TOPN, SL], F32)
            nc.sync.dma_start(
                out=sb_oh96[:],
                in_=_dram_ap(dram_oh, [[TOPN * SL, BL], [0, C], [1, TOPN * SL]]),
            )
            sb_simval = small.tile([PC, TOPN], F32, tag="simval")
            sb_scr = poolE.tile([PC, TOPN, SL], F32)
            nc.vector.tensor_tensor(
                out=sb_scr[:],
                in0=_ap(sb_sim1f, [[SL, PC], [0, TOPN], [1, SL]]),
                in1=sb_oh96[:], op=mybir.AluOpType.mult,
            )
            sb_scr2 = poolE.tile([PC, SL], F32)
            for n in range(TOPN):
                nc.scalar.activation(
                    out=sb_scr2[:], in_=sb_scr[:, n, :],
                    func=mybir.ActivationFunctionType.Identity,
                    accum_out=sb_simval[:, n : n + 1],
                )
            if KSTEP <= 3:
                return
            sb_rowf = small.tile([PC, TOPN], F32, tag="rowf")
            nc.vector.tensor_tensor(
                out=sb_rowf[:], in0=sb_simval[:],
                in1=sb_rowbase.to_broadcast([PC, TOPN]), op=mybir.AluOpType.add,
            )
            if "dbg_simval" in outs:
                nc.sync.dma_start(out=outs["dbg_simval"], in_=sb_simval[:])
            if "dbg_rowf" in outs:
                nc.sync.dma_start(out=outs["dbg_rowf"], in_=sb_rowf[:])
            sb_rowi16 = small.tile([128, TOPN], I16, tag="rowi16")
            nc.vector.memset(sb_rowi16[:], 0)
            nc.vector.tensor_copy(out=sb_rowi16[0:PC, :], in_=sb_rowf[:])

            # dram_idx flat [384] in gather order: entry i=128*j+p  (p>=96 -> 0)
            dram_idx = nc.dram_tensor("dram_idx", [NIDX], I16)
            nc.gpsimd.dma_start(
                out=_dram_ap(dram_idx, [[1, 128], [128, TOPN]]), in_=sb_rowi16[:]
            )
            if KSTEP <= 4:
                return
            # idxbuf[p16 + 16*g, col] = dram_idx[col*16 + p16], replicated to
            # all 8 Q7-core partition groups (HW reads per-core groups).
            for g in range(8):
                nc.gpsimd.dma_start(
                    out=sb_idxbuf[16 * g : 16 * (g + 1)],
                    in_=_dram_ap(dram_idx, [[1, 16], [16, NIDX // 16]]),
                )

        if KPHASE <= 5:
            return

        # ---------------- Phase F: gather + tgt_k + candidate logits
        with tc.tile_pool(name="phaseF", bufs=1) as poolF:
            sb_G = poolF.tile([128, TOPN, OBJ_D], F32)
            for j in range(TOPN):
                nc.gpsimd.dma_gather(
                    out_ap=sb_G[:, j : j + 1, :],
                    in_ap=ins["objfeat"],
                    idxs_ap=sb_idxbuf[:, 8 * j : 8 * (j + 1)],
                    num_idxs=128,
                    num_idxs_reg=128,
                    elem_size=OBJ_D,
                )
            if "dbg_gsum" in outs:
                sb_gsum = small.tile([128, TOPN], F32, tag="gsum")
                nc.vector.tensor_reduce(
                    out=sb_gsum[:], in_=sb_G[:], axis=mybir.AxisListType.X,
                    op=mybir.AluOpType.add,
                )
                nc.gpsimd.dma_start(out=outs["dbg_gsum"], in_=sb_gsum[:])

            # tgt_k tiles kept in SBUF (fp16): feat part [8, 2560], obj [8, 6144]
            sb_tkf = poolF.tile([BL, 2560], BF16)
            sb_tko = poolF.tile([BL, TOPN * OBJ_D], BF16)

            def tk_tiles(src_list, dst, cnt):
                for n in range(cnt):
                    wt = poolWk.tile([128, KH, NT_K], BF16, tag="wk", name="wt")
                    nc.sync.dma_start(out=wt[:], in_=src_list[n])
                    ps_tk = sm_tile([BL, NT_K])
                    for k in range(KH):
                        _mm(nc, ps_tk[:], sb_htT[:, k, :], wt[:, k, :],
                            k == 0, k == KH - 1)
                    nc.vector.tensor_copy(
                        out=dst[:, NT_K * n : NT_K * (n + 1)], in_=ps_tk[:]
                    )

            tk_tiles(ins["Wk_feat"], sb_tkf, 5)
            tk_tiles(ins["Wk_obj"], sb_tko, TOPN * OBJ_D // NT_K)

            # dots: 17 col-tiles of 512 (5 feat + 12 obj); T built on the fly by
            # PE broadcast (onehot12^T @ tk_slice -> psum [PC, 512])
            NTOT = 5 + TOPN * OBJ_D // NT_K
            sb_dots = small.tile([PC, NTOT], F32, tag="dots")
            sb_pd = poolF.tile([PC, NT_K], F32)
            sb_pr = poolF.tile([PC, NT_K], F32)
            widths_feat = [512, 512, 512, 512, 4]  # candf has 2052 cols
            for t in range(5):
                w = widths_feat[t]
                ps_bc = psum_bc.tile([PC, NT_K], F32, tag="bc", name="ps_bc")
                _mm(nc, ps_bc[:, 0:w], sb_oh12[:], sb_tkf[:, NT_K * t : NT_K * t + w],
                    True, True)
                nc.vector.tensor_tensor(
                    out=sb_pd[:, 0:w], in0=sb_candf[:, NT_K * t : NT_K * t + w],
                    in1=ps_bc[:, 0:w], op=mybir.AluOpType.mult,
                )
                nc.scalar.activation(
                    out=sb_pr[:, 0:w], in_=sb_pd[:, 0:w],
                    func=mybir.ActivationFunctionType.Identity,
                    accum_out=sb_dots[:, t : t + 1],
                )
            Gf = _ap(sb_G, [[TOPN * OBJ_D, PC], [1, TOPN * OBJ_D]])  # [PC, 6144]
            for t in range(TOPN * OBJ_D // NT_K):
                ps_bc = psum_bc.tile([PC, NT_K], F32, tag="bc", name="ps_bc")
                _mm(nc, ps_bc[:], sb_oh12[:], sb_tko[:, NT_K * t : NT_K * (t + 1)],
                    True, True)
                nc.vector.tensor_tensor(
                    out=sb_pd[:],
                    in0=bass.AP(tensor=Gf.tensor,
                                offset=Gf.offset + NT_K * t,
                                ap=[[TOPN * OBJ_D, PC], [1, NT_K]]),
                    in1=ps_bc[:], op=mybir.AluOpType.mult,
                )
                nc.scalar.activation(
                    out=sb_pr[:], in_=sb_pd[:],
                    func=mybir.ActivationFunctionType.Identity,
                    accum_out=sb_dots[:, 5 + t : 6 + t],
                )
            sb_logitF = small.tile([PC, 1], F32, tag="logitF")
            nc.vector.tensor_reduce(
                out=sb_logitF[:], in_=sb_dots[:], axis=mybir.AxisListType.X,
                op=mybir.AluOpType.add,
            )
            nc.gpsimd.dma_start(
                out=bass.AP(tensor=outs["out_logit"].tensor,
                            offset=outs["out_logit"].offset, ap=[[C, BL], [1, C]]),
                in_=sb_logitF[:],
            )


# ------------------------------------------------------------------ host side


def _stage_core(i, a):
    """Build the per-core input map (host-side reshapes only)."""
    bsl = slice(BL * i, BL * (i + 1))
    f32 = np.float32

    def chunkT(mat_t, kchunks, n):
        # mat_t [K, n] -> [128, kchunks, n]
        return np.ascontiguousarray(
            mat_t.reshape(kchunks, 128, n).transpose(1, 0, 2)
        ).astype(f32)

    m = {}
    m["eye8"] = np.eye(8, dtype=f32)
    act = a["action"][bsl]
    m["actionT5"] = np.concatenate([act.T, np.ones((1, BL), f32)], 0).astype(f32)
    m["WembT5"] = np.concatenate([a["W_emb"].T, a["b_emb"][None, :]], 0).astype(f32)
    m["h1T"] = chunkT(a["prev_h1"][bsl].T, KH, BL)
    m["c0"] = a["c_0"][bsl].astype(f32)

    feat = a["feature"][bsl]  # [BL, OBJ, FEAT]
    fpad = np.zeros((BL, OBJ, FEATP), f32)
    fpad[:, :, :FEAT] = feat
    m["featT"] = chunkT(fpad.transpose(2, 0, 1).reshape(FEATP, BS_F), KF, BS_F)

    wf = np.zeros((FEATP, H), f32)
    wf[:FEAT] = a["W_in_feat"]
    m["WfT"] = chunkT(wf.T, KH, FEATP)

    gperm = np.r_[0:H, H : 2 * H, 3 * H : 4 * H, 2 * H : 3 * H]  # [i,f,o,g]
    w_ih = np.asarray(a["W_ih"])[gperm]
    w_hh = np.asarray(a["W_hh"])[gperm]
    bias_row = (np.asarray(a["b_ih"]) + np.asarray(a["b_hh"]))[gperm].astype(
        f32)[None, :]
    m["Wih_emb65"] = np.concatenate([w_ih[:, :E].T, bias_row], 0).astype(f32)

    wihf = np.zeros((FEATP, 4 * H), f32)
    wihf[:FEAT] = w_ih[:, E:].T  # [2052, 2048]
    m["Wih_feat"] = [
        np.ascontiguousarray(
            wihf[:, NT_IH * n : NT_IH * (n + 1)]
            .reshape(KF, 128, NT_IH).transpose(1, 0, 2)
        )
        for n in range(4 * H // NT_IH)
    ]
    whh = w_hh.T.astype(f32)  # [512, 2048]
    m["Whh"] = [
        np.ascontiguousarray(
            whh[:, NT_IH * n : NT_IH * (n + 1)]
            .reshape(KH, 128, NT_IH).transpose(1, 0, 2)
        )
        for n in range(4 * H // NT_IH)
    ]

    m["ctxT"] = chunkT(
        a["ctx"][bsl].transpose(2, 0, 1).reshape(H, BS_C).astype(f32), KH, BS_C
    )
    m["WcT"] = chunkT(a["W_in_ctx"].T.astype(f32), KH, H)
    m["WoT"] = chunkT(a["W_out_ctx"].T.astype(f32), 2 * KH, H)
    m["maskadd"] = np.where(a["ctx_mask"][bsl], f32(-1e30), f32(0)).astype(f32)
    m["landmark"] = a["landmark_mask"][bsl].reshape(BL, SL).astype(f32)
    m["iota640"] = np.broadcast_to(np.arange(SL, dtype=f32), (PC, SL)).copy()
    m["sim1f"] = np.asarray(a["sim_matrix"])[1][bsl].reshape(PC, SL).astype(f32)
    m["rowbase"] = (np.arange(PC, dtype=f32) * OBJ)[:, None].copy()

    wkf = np.zeros((2560, H), f32)
    wkf[:FEAT] = a["W_in_cand"][:FEAT]
    wkfT = wkf.T  # [512, 2560]
    m["Wk_feat"] = [
        np.ascontiguousarray(
            wkfT[:, NT_K * n : NT_K * (n + 1)].reshape(KH, 128, NT_K).transpose(1, 0, 2)
        )
        for n in range(5)
    ]
    wkoT = np.ascontiguousarray(a["W_in_cand"][FEAT:].T)  # [512, 6144]
    m["Wk_obj"] = [
        np.ascontiguousarray(
            wkoT[:, NT_K * n : NT_K * (n + 1)].reshape(KH, 128, NT_K).transpose(1, 0, 2)
        )
        for n in range(TOPN * OBJ_D // NT_K)
    ]
    m["candf"] = a["cand_feat"][bsl].reshape(PC, FEAT).astype(f32)
    oh12 = np.zeros((BL, PC), f32)
    for b in range(BL):
        oh12[b, C * b : C * (b + 1)] = 1.0
    m["onehot12"] = oh12
    m["onehot12f"] = oh12
    m["ones128"] = np.ones((1, 128), f32)
    m["objfeat"] = np.ascontiguousarray(
        a["candidate_obj_feat"][bsl].reshape(NROW, OBJ_D)
    ).astype(f32)
    return m


_IN_SPECS = {
    "eye8": ([8, 8], F32),
    "actionT5": ([5, BL], F32),
    "WembT5": ([5, E], F32),
    "h1T": ([128, KH, BL], BF16),
    "c0": ([BL, H], F32),
    "featT": ([128, KF, BS_F], BF16),
    "WfT": ([128, KH, FEATP], BF16),
    "Wih_emb65": ([E + 1, 4 * H], BF16),
    "ctxT": ([128, KH, BS_C], BF16),
    "WcT": ([128, KH, H], BF16),
    "WoT": ([128, 2 * KH, H], BF16),
    "maskadd": ([BL, S], F32),
    "landmark": ([BL, SL], F32),
    "iota640": ([PC, SL], F32),
    "sim1f": ([PC, SL], F32),
    "rowbase": ([PC, 1], F32),
    "candf": ([PC, FEAT], F32),
    "onehot12": ([BL, PC], BF16),
    "onehot12f": ([BL, PC], F32),
    "ones128": ([1, 128], BF16),
    "objfeat": ([NROW, OBJ_D], F32),
}
_IN_LISTS = {
    "Wih_feat": (4 * H // NT_IH, [128, KF, NT_IH], BF16),
    "Whh": (4 * H // NT_IH, [128, KH, NT_IH], BF16),
    "Wk_feat": (5, [128, KH, NT_K], BF16),
    "Wk_obj": (TOPN * OBJ_D // NT_K, [128, KH, NT_K], BF16),
}
_BF16_INPUTS = {"h1T", "featT", "WfT", "Wih_emb65", "ctxT", "WcT", "WoT",
                "Wih_feat", "Whh", "Wk_feat", "Wk_obj", "onehot12", "ones128"}
_OUT_SPECS = {
    "out_h1": [BL, H],
    "out_c1": [BL, H],
    "out_htilde": [BL, H],
    "out_attn": [BL, S],
    "out_logit": [BL, C],
}
if os.environ.get("KDEBUG") == "1":
    _OUT_SPECS.update({
        "dbg_i3": [PC, TOPN],
        "dbg_simval": [PC, TOPN],
        "dbg_rowf": [PC, TOPN],
        "dbg_gsum": [128, TOPN],
    })


def compile_kernel():
    global _COMPILED
    if _COMPILED is not None:
        return _COMPILED
    nc = bacc.Bacc("TRN2", target_bir_lowering=False, debug=False,
                   num_devices=NCORES)
    ins = {}
    for name, (shape, dt) in _IN_SPECS.items():
        ins[name] = nc.dram_tensor(name, shape, dt, kind="ExternalInput").ap()
    for name, (cnt, shape, dt) in _IN_LISTS.items():
        ins[name] = [
            nc.dram_tensor(f"{name}_{j}", shape, dt, kind="ExternalInput").ap()
            for j in range(cnt)
        ]
    outs = {
        name: nc.dram_tensor(name, shape, F32, kind="ExternalOutput").ap()
        for name, shape in _OUT_SPECS.items()
    }
    with tile.TileContext(nc) as tc:
        build_kernel(tc, ins, outs)
    nc.compile()
    _COMPILED = nc
    return nc


def make_in_maps(inputs):
    in_maps = []
    for i in range(NCORES):
        m = _stage_core(i, inputs)
        flat = {}
        for k, v in m.items():
            dt16 = k in _BF16_INPUTS
            if isinstance(v, list):
                for j, arr in enumerate(v):
                    if dt16:
                        arr = arr.astype(np.float16)
                    flat[f"{k}_{j}"] = np.ascontiguousarray(arr)
            else:
                if dt16:
                    v = v.astype(np.float16)
                flat[k] = np.ascontiguousarray(v)
        in_maps.append(flat)
    return in_maps


def kernel(**inputs):
    inputs = {k: np.asarray(v) for k, v in inputs.items()}
    nc = compile_kernel()
    in_maps = make_in_maps(inputs)
    res = run_bass_kernel_spmd(nc, in_maps, core_ids=list(range(NCORES)))
    r = res.results
    h_1 = np.concatenate([r[i]["out_h1"] for i in range(NCORES)], 0)
    c_1 = np.concatenate([r[i]["out_c1"] for i in range(NCORES)], 0)
    logit = np.concatenate([r[i]["out_logit"] for i in range(NCORES)], 0)
    h_tilde = np.concatenate([r[i]["out_htilde"] for i in range(NCORES)], 0)
    ctx_attn = np.concatenate([r[i]["out_attn"] for i in range(NCORES)], 0)
    return (h_1, c_1, logit, h_tilde, ctx_attn)


if __name__ == "__main__":
    compile_kernel()
    print("compiled OK")


# revision 33
# speedup vs baseline: 1.0174x; 1.0174x over previous
"""Trainium2 Bass kernel for nn_ConfigurationLXMERTDecoder (retrieval_knn).

Data-parallel across 8 NeuronCores: batch 64 -> 8 per core. All compute on
device; host only reshapes/shards inputs into the SBUF-friendly layouts the
kernel DMAs in, and concatenates the per-core outputs.

Outputs (matching reference): (h_1, c_1, logit, h_tilde, ctx_attn)
"""

import os
import sys

sys.path.insert(0, "/opt/trn_rl_repo")

from contextlib import ExitStack

import numpy as np

import concourse.bass as bass
import concourse.bacc as bacc
import concourse.tile as tile
from concourse import mybir
from concourse.bass_utils import run_bass_kernel_spmd

# model dims
B, S, L, C, OBJ = 64, 80, 8, 12, 36
AFS, E, H, FEAT = 4, 64, 512, 2052
TOPN = 3
OBJ_D = 2048
SL = S * L  # 640

NCORES = 8
BL = B // NCORES  # 8 batch per core
FEATP = 2176  # FEAT padded to 17*128
KF = FEATP // 128  # 17
KH = H // 128  # 4
BS_F = BL * OBJ  # 288  (b,s) pairs for feature attention
BS_C = BL * S  # 640  (b,s) pairs for ctx attention
PC = BL * C  # 96   (b,c) pairs
NROW = PC * OBJ  # 3456 obj rows per core
KCAND = 8704  # 8196 padded to 17*512
NIDX = 384  # gather slots (3 cols x 128 partitions)
NT_IH = 512  # gates col-tile
NT_K = 512  # tgt_k col-tile

F32 = mybir.dt.float32
F32R = mybir.dt.float32r
BF16 = mybir.dt.float16  # 16-bit matmul dtype (fp16: 10-bit mantissa)
I16 = mybir.dt.int16
U32 = mybir.dt.uint32

USE_F32R = False  # stream fp32 matmul operands as float32r (full-rate) via bitcast
KPHASE = int(os.environ.get("KPHASE", "9"))  # debug: stop after phase N
KSTEP = int(os.environ.get("KSTEP", "9"))  # debug: stop mid-phase-E

_COMPILED = None


def _ap(t, ap_dims, offset=0):
    base = t[:]
    return bass.AP(tensor=base.tensor, offset=base.offset + offset, ap=ap_dims)


def _dram_ap(d, ap_dims, offset=0):
    return bass.AP(tensor=d.ap().tensor, offset=offset, ap=ap_dims)


def _mm(nc, out, lhsT, rhs, start, stop):
    if USE_F32R:
        lhsT = lhsT.bitcast(F32R)
        rhs = rhs.bitcast(F32R)
    nc.tensor.matmul(out=out, lhsT=lhsT, rhs=rhs, start=start, stop=stop)


def _softmax(nc, pool, out, logits, n):
    """out = softmax(logits) along free dim (logits bounded: no max-subtract)."""
    p = logits.shape[0]
    ex = pool.tile([p, n], F32, tag="sm_exp", name="ex")
    ssum = pool.tile([p, 1], F32, tag="sm_sum", name="ssum")
    nc.scalar.activation(
        out=ex[:], in_=logits[:], func=mybir.ActivationFunctionType.Exp,
        accum_out=ssum[:],
    )
    rsum = pool.tile([p, 1], F32, tag="sm_rsum", name="rsum")
    nc.vector.reciprocal(out=rsum[:], in_=ssum[:])
    nc.vector.tensor_tensor(
        out=out[:], in0=ex[:], in1=rsum.to_broadcast([p, n]), op=mybir.AluOpType.mult,
    )


def build_kernel(tc, ins, outs):
    nc = tc.nc
    ctx = ExitStack()
    with ctx:
        persist = ctx.enter_context(tc.tile_pool(name="persist", bufs=1))
        small = ctx.enter_context(tc.tile_pool(name="small", bufs=1))
        psum = ctx.enter_context(tc.tile_pool(name="psum", bufs=1, space="PSUM"))
        psum_s = ctx.enter_context(tc.tile_pool(name="psum_s", bufs=2, space="PSUM"))
        psum_bc = ctx.enter_context(tc.tile_pool(name="psum_bc", bufs=2,
                                                 space="PSUM"))
        # weight streaming pools (kernel-lifetime so DMA prefetch is unblocked)
        poolWih = ctx.enter_context(tc.tile_pool(name="wih", bufs=2))
        poolWhh = ctx.enter_context(tc.tile_pool(name="whh", bufs=2))
        poolWk = ctx.enter_context(tc.tile_pool(name="wk", bufs=4))
        poolWf = ctx.enter_context(tc.tile_pool(name="wf", bufs=2))

        def big_tile(shape):
            return psum.tile(shape, F32, tag="big", name="ps_big")

        def sm_tile(shape):
            return psum_s.tile(shape, F32, tag="sm", name="ps_sm")

        # ---- persistent small loads
        sb_eye8 = persist.tile([8, 8], F32)
        nc.scalar.dma_start(out=sb_eye8[:], in_=ins["eye8"])
        sb_actionT5 = persist.tile([5, BL], F32)
        nc.scalar.dma_start(out=sb_actionT5[:], in_=ins["actionT5"])
        sb_WembT5 = persist.tile([5, E], F32)
        nc.scalar.dma_start(out=sb_WembT5[:], in_=ins["WembT5"])
        sb_h1T = persist.tile([128, KH, BL], BF16)
        nc.scalar.dma_start(out=sb_h1T[:], in_=ins["h1T"])
        sb_c0 = persist.tile([BL, H], F32)
        nc.scalar.dma_start(out=sb_c0[:], in_=ins["c0"])
        sb_maskadd = persist.tile([BL, S], F32)
        nc.scalar.dma_start(out=sb_maskadd[:], in_=ins["maskadd"])
        sb_landmark = persist.tile([BL, SL], F32)
        nc.scalar.dma_start(out=sb_landmark[:], in_=ins["landmark"])
        sb_iota = persist.tile([PC, SL], F32)
        nc.scalar.dma_start(out=sb_iota[:], in_=ins["iota640"])
        sb_sim1f = persist.tile([PC, SL], F32)
        nc.scalar.dma_start(out=sb_sim1f[:], in_=ins["sim1f"])
        sb_rowbase = persist.tile([PC, 1], F32)
        nc.scalar.dma_start(out=sb_rowbase[:], in_=ins["rowbase"])
        sb_candf = persist.tile([PC, FEAT], F32)
        nc.scalar.dma_start(out=sb_candf[:], in_=ins["candf"])
        sb_Wih_emb = persist.tile([E + 1, 4 * H], BF16)
        nc.scalar.dma_start(out=sb_Wih_emb[:], in_=ins["Wih_emb65"])

        sb_afT = persist.tile([128, KF, BL], BF16)  # attn_feat^T chunks (lhsT)
        sb_oh12 = persist.tile([BL, PC], BF16)  # onehot12[b, 12b+c] = 1
        nc.gpsimd.dma_start(out=sb_oh12[:], in_=ins["onehot12"])
        sb_ones128 = persist.tile([1, 128], BF16)
        nc.gpsimd.dma_start(out=sb_ones128[:], in_=ins["ones128"])
        sb_oh12f = persist.tile([BL, PC], F32)
        nc.gpsimd.dma_start(out=sb_oh12f[:], in_=ins["onehot12f"])

        # ---------------- Phase A: action embedding  embT [E, BL]
        ps_emb = sm_tile([E, BL])
        _mm(nc, ps_emb[:], sb_WembT5[:], sb_actionT5[:], True, True)
        sb_embT65 = persist.tile([E + 1, BL], BF16)
        nc.scalar.activation(
            out=sb_embT65[0:E, :], in_=ps_emb[:],
            func=mybir.ActivationFunctionType.Tanh,
        )
        nc.vector.memset(sb_embT65[E : E + 1, :], 1.0)
        if KPHASE <= 1:
            return

        # ---------------- Phase B: feature attention
        with tc.tile_pool(name="phaseB", bufs=1) as poolB:
            sb_featT = poolB.tile([128, KF, BS_F], BF16)
            nc.sync.dma_start(out=sb_featT[:], in_=ins["featT"])

            # tgt_f = prev_h1 @ W_in_feat.T ; 5 col-tiles (4x512 + 128)
            sb_tgtf = poolB.tile([BL, FEATP], F32)
            for q in range(5):
                qn = 512 if q < 4 else 128
                wq = poolWf.tile([128, KH, 512], BF16, tag="wf", name="wf_q")
                nc.sync.dma_start(
                    out=wq[:, :, 0:qn],
                    in_=ins["WfT"][:, :, 512 * q : 512 * q + qn],
                )
                ps_q = sm_tile([BL, 512])
                for k in range(KH):
                    _mm(
                        nc, ps_q[:, 0:qn], sb_h1T[:, k, :],
                        wq[:, k, 0:qn], k == 0, k == KH - 1,
                    )
                nc.scalar.activation(
                    out=sb_tgtf[:, 512 * q : 512 * q + qn], in_=ps_q[:, 0:qn],
                    func=mybir.ActivationFunctionType.Copy,
                )

            # transpose -> tgt_f^T [128, KF, BL]
            ps_tfT = sm_tile([128, KF, BL])
            for t in range(KF):
                nc.tensor.transpose(
                    ps_tfT[:, t, :], sb_tgtf[:, 128 * t : 128 * (t + 1)], sb_eye8[:]
                )
            sb_tfT = poolB.tile([128, KF, BL], BF16)
            nc.vector.tensor_copy(out=sb_tfT[:], in_=ps_tfT[:])

            # logits: lf[b, (b',s)] block-diag useful  -> psum [BL, BS_F]
            ps_lf = sm_tile([BL, BS_F])
            for k in range(KF):
                _mm(nc, ps_lf[:], sb_tfT[:, k, :], sb_featT[:, k, :], k == 0, k == KF - 1)
            sb_lfull = small.tile([BL, BS_F], F32, tag="lfull")
            nc.scalar.activation(out=sb_lfull[:], in_=ps_lf[:],
                                 func=mybir.ActivationFunctionType.Copy)
            sb_lf = small.tile([BL, OBJ], F32, tag="lf")
            pit = sb_lfull[:].ap[0][0]
            nc.gpsimd.dma_start(
                out=sb_lf[:], in_=_ap(sb_lfull, [[pit + OBJ, BL], [1, OBJ]])
            )
            sb_attnf = small.tile([BL, OBJ], F32, tag="attnf")
            _softmax(nc, small, sb_attnf, sb_lf, OBJ)

            # attn_feat^T = sum_s attn[b,s] * featT[:, :, (b,s)]
            # broadcast attn to all 128 partitions via PE: ones^T @ attn_flat
            sb_attnf16 = small.tile([BL, OBJ], BF16, tag="attnf16")
            nc.vector.tensor_copy(out=sb_attnf16[:], in_=sb_attnf[:])
            sb_aflat = small.tile([1, BS_F], BF16, tag="aflat")
            nc.gpsimd.dma_start(out=sb_aflat[:], in_=sb_attnf16[:])
            ps_ab = psum_bc.tile([128, BS_F], F32, tag="bc", name="ps_ab")
            _mm(nc, ps_ab[:], sb_ones128[:], sb_aflat[:], True, True)
            nc.vector.tensor_tensor(
                out=sb_featT[:], in0=sb_featT[:],
                in1=_ap(ps_ab, [[BS_F, 128], [0, KF], [1, BS_F]]),
                op=mybir.AluOpType.mult,
            )
            sb_afTf = poolB.tile([128, KF, BL], F32)
            nc.vector.tensor_reduce(
                out=sb_afTf[:],
                in_=sb_featT[:].rearrange("p k (b s) -> p k b s", b=BL),
                axis=mybir.AxisListType.X, op=mybir.AluOpType.add,
            )
            nc.vector.tensor_copy(out=sb_afT[:], in_=sb_afTf[:])

        if KPHASE <= 2:
            return

        # ---------------- Phase C: LSTM gates + cell
        ps_gates = big_tile([BL, 4 * H])
        for n in range(4 * H // NT_IH):  # 4 tiles of 512
            cols = slice(NT_IH * n, NT_IH * (n + 1))
            _mm(nc, ps_gates[:, cols], sb_embT65[:], sb_Wih_emb[:, cols], True, False)
            wtf = poolWih.tile([128, KF, NT_IH], BF16, tag="wihf", name="wtf")
            nc.sync.dma_start(out=wtf[:], in_=ins["Wih_feat"][n])
            for k in range(KF):
                _mm(nc, ps_gates[:, cols], sb_afT[:, k, :], wtf[:, k, :], False, False)
            wth = poolWhh.tile([128, KH, NT_IH], BF16, tag="whh", name="wth")
            nc.sync.dma_start(out=wth[:], in_=ins["Whh"][n])
            for k in range(KH):
                _mm(
                    nc, ps_gates[:, cols], sb_h1T[:, k, :], wth[:, k, :],
                    False, k == KH - 1,
                )

        # gates in [i, f, g, o] order; per-bank activations fire as each
        # 512-col psum tile completes (bank-level deps)
        sb_sigi = small.tile([BL, H], F32, tag="sigi")
        sb_sigf = small.tile([BL, H], F32, tag="sigf")
        sb_gg = small.tile([BL, H], F32, tag="gg")
        sb_sigo = small.tile([BL, H], F32, tag="sigo")
        nc.scalar.activation(out=sb_sigi[:], in_=ps_gates[:, 0:H],
                             func=mybir.ActivationFunctionType.Sigmoid)
        nc.scalar.activation(out=sb_sigf[:], in_=ps_gates[:, H : 2 * H],
                             func=mybir.ActivationFunctionType.Sigmoid)
        nc.scalar.activation(out=sb_gg[:], in_=ps_gates[:, 2 * H : 3 * H],
                             func=mybir.ActivationFunctionType.Tanh)
        sb_c1 = persist.tile([BL, H], F32)
        sb_t1 = small.tile([BL, H], F32, tag="t1")
        nc.vector.tensor_tensor(out=sb_t1[:], in0=sb_sigf[:], in1=sb_c0[:],
                                op=mybir.AluOpType.mult)
        nc.vector.tensor_tensor(out=sb_gg[:], in0=sb_sigi[:], in1=sb_gg[:],
                                op=mybir.AluOpType.mult)
        nc.vector.tensor_tensor(out=sb_c1[:], in0=sb_t1[:], in1=sb_gg[:],
                                op=mybir.AluOpType.add)
        sb_tanhc1 = small.tile([BL, H], F32, tag="tanhc1")
        nc.scalar.activation(out=sb_tanhc1[:], in_=sb_c1[:],
                             func=mybir.ActivationFunctionType.Tanh)
        nc.scalar.activation(out=sb_sigo[:], in_=ps_gates[:, 3 * H : 4 * H],
                             func=mybir.ActivationFunctionType.Sigmoid)
        sb_h1 = persist.tile([BL, H], F32)
        nc.vector.tensor_tensor(out=sb_h1[:], in0=sb_sigo[:],
                                in1=sb_tanhc1[:], op=mybir.AluOpType.mult)
        nc.gpsimd.dma_start(out=outs["out_c1"], in_=sb_c1[:])
        nc.gpsimd.dma_start(out=outs["out_h1"], in_=sb_h1[:])

        # h1^T for later matmuls
        ps_h1T = sm_tile([128, KH, BL])
        for t in range(KH):
            nc.tensor.transpose(
                ps_h1T[:, t, :], sb_h1[:, 128 * t : 128 * (t + 1)], sb_eye8[:]
            )
        sb_h1T4 = persist.tile([128, KH, BL], BF16)
        nc.vector.tensor_copy(out=sb_h1T4[:], in_=ps_h1T[:])

        if KPHASE <= 3:
            return

        # ---------------- Phase D: ctx attention
        sb_attnc = persist.tile([BL, S], F32)
        sb_ht = persist.tile([BL, H], F32)
        with tc.tile_pool(name="phaseD", bufs=1) as poolD:
            sb_ctxT = poolD.tile([128, KH, BS_C], BF16)
            nc.sync.dma_start(out=sb_ctxT[:], in_=ins["ctxT"])
            sb_WcT = poolD.tile([128, KH, H], BF16)
            nc.scalar.dma_start(out=sb_WcT[:], in_=ins["WcT"])
            sb_WoT = poolD.tile([128, 2 * KH, H], BF16)
            nc.scalar.dma_start(out=sb_WoT[:], in_=ins["WoT"])

            # tgt_c^T directly: [d_tile, b] = sum_h WcT[h, d_tile] * h1T[h, b]
            ps_tcT = sm_tile([128, KH, BL])
            for m in range(KH):
                for k in range(KH):
                    _mm(nc, ps_tcT[:, m, :],
                        sb_WcT[:, k, 128 * m : 128 * (m + 1)],
                        sb_h1T4[:, k, :], k == 0, k == KH - 1)
            sb_tcT = small.tile([128, KH, BL], BF16, tag="tcT")
            nc.vector.tensor_copy(out=sb_tcT[:], in_=ps_tcT[:])

            ps_lc = big_tile([BL, BS_C])
            for k in range(KH):
                _mm(nc, ps_lc[:, 0:512], sb_tcT[:, k, :], sb_ctxT[:, k, 0:512],
                    k == 0, k == KH - 1)
            for k in range(KH):
                _mm(nc, ps_lc[:, 512:640], sb_tcT[:, k, :], sb_ctxT[:, k, 512:640],
                    k == 0, k == KH - 1)
            sb_lcfull = small.tile([BL, BS_C], F32, tag="lcfull")
            nc.scalar.activation(out=sb_lcfull[:], in_=ps_lc[:],
                                 func=mybir.ActivationFunctionType.Copy)
            sb_lc = small.tile([BL, S], F32, tag="lc")
            pitc = sb_lcfull[:].ap[0][0]
            nc.gpsimd.dma_start(
                out=sb_lc[:], in_=_ap(sb_lcfull, [[pitc + S, BL], [1, S]])
            )
            nc.vector.tensor_tensor(out=sb_lc[:], in0=sb_lc[:], in1=sb_maskadd[:],
                                    op=mybir.AluOpType.add)
            _softmax(nc, small, sb_attnc, sb_lc, S)
            nc.gpsimd.dma_start(out=outs["out_attn"], in_=sb_attnc[:])

            # wctx^T via weighted sum over s; PE broadcast in 2 b-aligned tiles
            sb_attnc16 = small.tile([BL, S], BF16, tag="attnc16")
            nc.vector.tensor_copy(out=sb_attnc16[:], in_=sb_attnc[:])
            sb_acflat = small.tile([1, BS_C], BF16, tag="acflat")
            nc.gpsimd.dma_start(out=sb_acflat[:], in_=sb_attnc16[:])
            for lo, w in ((0, 400), (400, 240)):
                ps_ac = psum_bc.tile([128, 512], F32, tag="bc", name="ps_ac")
                _mm(nc, ps_ac[:, 0:w], sb_ones128[:], sb_acflat[:, lo : lo + w],
                    True, True)
                nc.vector.tensor_tensor(
                    out=sb_ctxT[:, :, lo : lo + w],
                    in0=sb_ctxT[:, :, lo : lo + w],
                    in1=_ap(ps_ac, [[512, 128], [0, KH], [1, w]]),
                    op=mybir.AluOpType.mult,
                )
            sb_wcTf = small.tile([128, KH, BL], F32, tag="wcTf")
            nc.vector.tensor_reduce(
                out=sb_wcTf[:],
                in_=sb_ctxT[:].rearrange("p k (b s) -> p k b s", b=BL),
                axis=mybir.AxisListType.X, op=mybir.AluOpType.add,
            )
            sb_wcT = small.tile([128, KH, BL], BF16, tag="wcT")
            nc.vector.tensor_copy(out=sb_wcT[:], in_=sb_wcTf[:])

            ps_ht = sm_tile([BL, H])
            for k in range(2 * KH):
                lhsT = sb_wcT[:, k, :] if k < KH else sb_h1T4[:, k - KH, :]
                _mm(nc, ps_ht[:], lhsT, sb_WoT[:, k, :], k == 0, k == 2 * KH - 1)
            nc.scalar.activation(out=sb_ht[:], in_=ps_ht[:],
                                 func=mybir.ActivationFunctionType.Tanh)
            nc.gpsimd.dma_start(out=outs["out_htilde"], in_=sb_ht[:])

        # h_tilde^T for tgt_k
        ps_htT = sm_tile([128, KH, BL])
        for t in range(KH):
            nc.tensor.transpose(
                ps_htT[:, t, :], sb_ht[:, 128 * t : 128 * (t + 1)], sb_eye8[:]
            )
        sb_htT = persist.tile([128, KH, BL], BF16)
        nc.vector.tensor_copy(out=sb_htT[:], in_=ps_htT[:])

        if KPHASE <= 4:
            return

        # ---------------- Phase E: top-3 + sim gather indices
        sb_scores = small.tile([BL, SL], F32, tag="scores")
        nc.vector.tensor_tensor(
            out=sb_scores[:], in0=sb_landmark[:],
            in1=_ap(sb_attnc, [[S, BL], [1, S], [0, L]]),
            op=mybir.AluOpType.mult,
        )
        sb_v8 = small.tile([BL, 8], F32, tag="v8")
        nc.vector.max(out=sb_v8[:], in_=sb_scores[:])
        sb_i8 = small.tile([BL, 8], U32, tag="i8")
        nc.vector.max_index(out=sb_i8[:], in_max=sb_v8[:], in_values=sb_scores[:])
        if KSTEP <= 1:
            return
        sb_i3f = small.tile([BL, TOPN], F32, tag="i3f")
        nc.vector.tensor_copy(out=sb_i3f[:], in_=sb_i8[:, 0:TOPN])
        if "dbg_i3" in outs:
            nc.sync.dma_start(out=outs["dbg_i3"], in_=sb_i3f[:])
        sb_onehot = small.tile([BL, TOPN, SL], F32, tag="onehot")
        nc.vector.tensor_tensor(
            out=sb_onehot[:],
            in0=_ap(sb_iota, [[SL, BL], [0, TOPN], [1, SL]]),
            in1=_ap(sb_i3f, [[TOPN, BL], [1, TOPN], [0, SL]]),
            op=mybir.AluOpType.is_equal,
        )
        if KSTEP <= 2:
            return
        dram_oh = nc.dram_tensor("dram_oh", [BL, TOPN * SL], F32)
        nc.sync.dma_start(out=dram_oh.ap(), in_=sb_onehot[:])
        sb_idxbuf = persist.tile([128, NIDX // 16], I16)
        with tc.tile_pool(name="phaseE", bufs=1) as poolE:
            sb_oh96 = poolE.tile([PC, TOPN, SL], F32)
            nc.sync.dma_start(
                out=sb_oh96[:],
                in_=_dram_ap(dram_oh, [[TOPN * SL, BL], [0, C], [1, TOPN * SL]]),
            )
            sb_simval = small.tile([PC, TOPN], F32, tag="simval")
            sb_scr = poolE.tile([PC, TOPN, SL], F32)
            nc.vector.tensor_tensor(
                out=sb_scr[:],
                in0=_ap(sb_sim1f, [[SL, PC], [0, TOPN], [1, SL]]),
                in1=sb_oh96[:], op=mybir.AluOpType.mult,
            )
            sb_scr2 = poolE.tile([PC, SL], F32)
            for n in range(TOPN):
                nc.scalar.activation(
                    out=sb_scr2[:], in_=sb_scr[:, n, :],
                    func=mybir.ActivationFunctionType.Identity,
                    accum_out=sb_simval[:, n : n + 1],
                )
            if KSTEP <= 3:
                return
            sb_rowf = small.tile([PC, TOPN], F32, tag="rowf")
            nc.vector.tensor_tensor(
                out=sb_rowf[:], in0=sb_simval[:],
                in1=sb_rowbase.to_broadcast([PC, TOPN]), op=mybir.AluOpType.add,
            )
            if "dbg_simval" in outs:
                nc.sync.dma_start(out=outs["dbg_simval"], in_=sb_simval[:])
            if "dbg_rowf" in outs:
                nc.sync.dma_start(out=outs["dbg_rowf"], in_=sb_rowf[:])
            sb_rowi16 = small.tile([128, TOPN], I16, tag="rowi16")
            nc.vector.memset(sb_rowi16[:], 0)
            nc.vector.tensor_copy(out=sb_rowi16[0:PC, :], in_=sb_rowf[:])

            # dram_idx flat [384] in gather order: entry i=128*j+p  (p>=96 -> 0)
            dram_idx = nc.dram_tensor("dram_idx", [NIDX], I16)
            nc.gpsimd.dma_start(
                out=_dram_ap(dram_idx, [[1, 128], [128, TOPN]]), in_=sb_rowi16[:]
            )
            if KSTEP <= 4:
                return
            # idxbuf[p16 + 16*g, col] = dram_idx[col*16 + p16], replicated to
            # all 8 Q7-core partition groups (HW reads per-core groups).
            for g in range(8):
                nc.gpsimd.dma_start(
                    out=sb_idxbuf[16 * g : 16 * (g + 1)],
                    in_=_dram_ap(dram_idx, [[1, 16], [16, NIDX // 16]]),
                )

        if KPHASE <= 5:
            return

        # ---------------- Phase F: gather + tgt_k + candidate logits
        with tc.tile_pool(name="phaseF", bufs=1) as poolF:
            sb_G = poolF.tile([128, TOPN, OBJ_D], F32)
            for j in range(TOPN):
                nc.gpsimd.dma_gather(
                    out_ap=sb_G[:, j : j + 1, :],
                    in_ap=ins["objfeat"],
                    idxs_ap=sb_idxbuf[:, 8 * j : 8 * (j + 1)],
                    num_idxs=128,
                    num_idxs_reg=128,
                    elem_size=OBJ_D,
                )
            if "dbg_gsum" in outs:
                sb_gsum = small.tile([128, TOPN], F32, tag="gsum")
                nc.vector.tensor_reduce(
                    out=sb_gsum[:], in_=sb_G[:], axis=mybir.AxisListType.X,
                    op=mybir.AluOpType.add,
                )
                nc.gpsimd.dma_start(out=outs["dbg_gsum"], in_=sb_gsum[:])

            # tgt_k tiles kept in SBUF (fp16): feat part [8, 2560], obj [8, 6144]
            sb_tkf = poolF.tile([BL, 2560], BF16)
            sb_tko = poolF.tile([BL, TOPN * OBJ_D], BF16)

            def tk_tiles(src_list, dst, cnt):
                for n in range(cnt):
                    wt = poolWk.tile([128, KH, NT_K], BF16, tag="wk", name="wt")
                    nc.sync.dma_start(out=wt[:], in_=src_list[n])
                    ps_tk = sm_tile([BL, NT_K])
                    for k in range(KH):
                        _mm(nc, ps_tk[:], sb_htT[:, k, :], wt[:, k, :],
                            k == 0, k == KH - 1)
                    nc.vector.tensor_copy(
                        out=dst[:, NT_K * n : NT_K * (n + 1)], in_=ps_tk[:]
                    )

            tk_tiles(ins["Wk_feat"], sb_tkf, 5)
            tk_tiles(ins["Wk_obj"], sb_tko, TOPN * OBJ_D // NT_K)

            # dots: 17 col-tiles of 512 (5 feat + 12 obj); T built on the fly by
            # PE broadcast (onehot12^T @ tk_slice -> psum [PC, 512])
            NTOT = 5 + TOPN * OBJ_D // NT_K
            sb_dots = small.tile([PC, NTOT], F32, tag="dots")
            sb_pd = poolF.tile([PC, NT_K], F32)
            sb_pr = poolF.tile([PC, NT_K], F32)
            widths_feat = [512, 512, 512, 512, 4]  # candf has 2052 cols
            for t in range(5):
                w = widths_feat[t]
                ps_bc = psum_bc.tile([PC, NT_K], F32, tag="bc", name="ps_bc")
                _mm(nc, ps_bc[:, 0:w], sb_oh12[:], sb_tkf[:, NT_K * t : NT_K * t + w],
                    True, True)
                nc.vector.tensor_tensor(
                    out=sb_pd[:, 0:w], in0=sb_candf[:, NT_K * t : NT_K * t + w],
                    in1=ps_bc[:, 0:w], op=mybir.AluOpType.mult,
                )
                nc.scalar.activation(
                    out=sb_pr[:, 0:w], in_=sb_pd[:, 0:w],
                    func=mybir.ActivationFunctionType.Identity,
                    accum_out=sb_dots[:, t : t + 1],
                )
            Gf = _ap(sb_G, [[TOPN * OBJ_D, PC], [1, TOPN * OBJ_D]])  # [PC, 6144]
            for t in range(TOPN * OBJ_D // NT_K):
                ps_bc = psum_bc.tile([PC, NT_K], F32, tag="bc", name="ps_bc")
                _mm(nc, ps_bc[:], sb_oh12[:], sb_tko[:, NT_K * t : NT_K * (t + 1)],
                    True, True)
                nc.vector.tensor_tensor(
                    out=sb_pd[:],
                    in0=bass.AP(tensor=Gf.tensor,
                                offset=Gf.offset + NT_K * t,
                                ap=[[TOPN * OBJ_D, PC], [1, NT_K]]),
                    in1=ps_bc[:], op=mybir.AluOpType.mult,
                )
                nc.scalar.activation(
                    out=sb_pr[:], in_=sb_pd[:],
                    func=mybir.ActivationFunctionType.Identity,
                    accum_out=sb_dots[:, 5 + t : 6 + t],
                )
            sb_logitF = small.tile([PC, 1], F32, tag="logitF")
            nc.vector.tensor_reduce(
                out=sb_logitF[:], in_=sb_dots[:], axis=mybir.AxisListType.X,
                op=mybir.AluOpType.add,
            )
            nc.gpsimd.dma_start(
                out=bass.AP(tensor=outs["out_logit"].tensor,
                            offset=outs["out_logit"].offset, ap=[[C, BL], [1, C]]),
                in_=sb_logitF[:],
            )


# ------------------------------------------------------------------ host side


def _stage_core(i, a):
    """Build the per-core input map (host-side reshapes only)."""
    bsl = slice(BL * i, BL * (i + 1))
    f32 = np.float32

    def chunkT(mat_t, kchunks, n):
        # mat_t [K, n] -> [128, kchunks, n]
        return np.ascontiguousarray(
            mat_t.reshape(kchunks, 128, n).transpose(1, 0, 2)
        ).astype(f32)

    m = {}
    m["eye8"] = np.eye(8, dtype=f32)
    act = a["action"][bsl]
    m["actionT5"] = np.concatenate([act.T, np.ones((1, BL), f32)], 0).astype(f32)
    m["WembT5"] = np.concatenate([a["W_emb"].T, a["b_emb"][None, :]], 0).astype(f32)
    m["h1T"] = chunkT(a["prev_h1"][bsl].T, KH, BL)
    m["c0"] = a["c_0"][bsl].astype(f32)

    feat = a["feature"][bsl]  # [BL, OBJ, FEAT]
    fpad = np.zeros((BL, OBJ, FEATP), f32)
    fpad[:, :, :FEAT] = feat
    m["featT"] = chunkT(fpad.transpose(2, 0, 1).reshape(FEATP, BS_F), KF, BS_F)

    wf = np.zeros((FEATP, H), f32)
    wf[:FEAT] = a["W_in_feat"]
    m["WfT"] = chunkT(wf.T, KH, FEATP)

    w_ih = np.asarray(a["W_ih"])
    w_hh = np.asarray(a["W_hh"])
    bias_row = (np.asarray(a["b_ih"]) + np.asarray(a["b_hh"])).astype(
        f32)[None, :]
    m["Wih_emb65"] = np.concatenate([w_ih[:, :E].T, bias_row], 0).astype(f32)

    wihf = np.zeros((FEATP, 4 * H), f32)
    wihf[:FEAT] = w_ih[:, E:].T  # [2052, 2048]
    m["Wih_feat"] = [
        np.ascontiguousarray(
            wihf[:, NT_IH * n : NT_IH * (n + 1)]
            .reshape(KF, 128, NT_IH).transpose(1, 0, 2)
        )
        for n in range(4 * H // NT_IH)
    ]
    whh = w_hh.T.astype(f32)  # [512, 2048]
    m["Whh"] = [
        np.ascontiguousarray(
            whh[:, NT_IH * n : NT_IH * (n + 1)]
            .reshape(KH, 128, NT_IH).transpose(1, 0, 2)
        )
        for n in range(4 * H // NT_IH)
    ]

    m["ctxT"] = chunkT(
        a["ctx"][bsl].transpose(2, 0, 1).reshape(H, BS_C).astype(f32), KH, BS_C
    )
    m["WcT"] = chunkT(a["W_in_ctx"].T.astype(f32), KH, H)
    m["WoT"] = chunkT(a["W_out_ctx"].T.astype(f32), 2 * KH, H)
    m["maskadd"] = np.where(a["ctx_mask"][bsl], f32(-1e30), f32(0)).astype(f32)
    m["landmark"] = a["landmark_mask"][bsl].reshape(BL, SL).astype(f32)
    m["iota640"] = np.broadcast_to(np.arange(SL, dtype=f32), (PC, SL)).copy()
    m["sim1f"] = np.asarray(a["sim_matrix"])[1][bsl].reshape(PC, SL).astype(f32)
    m["rowbase"] = (np.arange(PC, dtype=f32) * OBJ)[:, None].copy()

    wkf = np.zeros((2560, H), f32)
    wkf[:FEAT] = a["W_in_cand"][:FEAT]
    wkfT = wkf.T  # [512, 2560]
    m["Wk_feat"] = [
        np.ascontiguousarray(
            wkfT[:, NT_K * n : NT_K * (n + 1)].reshape(KH, 128, NT_K).transpose(1, 0, 2)
        )
        for n in range(5)
    ]
    wkoT = np.ascontiguousarray(a["W_in_cand"][FEAT:].T)  # [512, 6144]
    m["Wk_obj"] = [
        np.ascontiguousarray(
            wkoT[:, NT_K * n : NT_K * (n + 1)].reshape(KH, 128, NT_K).transpose(1, 0, 2)
        )
        for n in range(TOPN * OBJ_D // NT_K)
    ]
    m["candf"] = a["cand_feat"][bsl].reshape(PC, FEAT).astype(f32)
    oh12 = np.zeros((BL, PC), f32)
    for b in range(BL):
        oh12[b, C * b : C * (b + 1)] = 1.0
    m["onehot12"] = oh12
    m["onehot12f"] = oh12
    m["ones128"] = np.ones((1, 128), f32)
    m["objfeat"] = np.ascontiguousarray(
        a["candidate_obj_feat"][bsl].reshape(NROW, OBJ_D)
    ).astype(f32)
    return m


_IN_SPECS = {
    "eye8": ([8, 8], F32),
    "actionT5": ([5, BL], F32),
    "WembT5": ([5, E], F32),
    "h1T": ([128, KH, BL], BF16),
    "c0": ([BL, H], F32),
    "featT": ([128, KF, BS_F], BF16),
    "WfT": ([128, KH, FEATP], BF16),
    "Wih_emb65": ([E + 1, 4 * H], BF16),
    "ctxT": ([128, KH, BS_C], BF16),
    "WcT": ([128, KH, H], BF16),
    "WoT": ([128, 2 * KH, H], BF16),
    "maskadd": ([BL, S], F32),
    "landmark": ([BL, SL], F32),
    "iota640": ([PC, SL], F32),
    "sim1f": ([PC, SL], F32),
    "rowbase": ([PC, 1], F32),
    "candf": ([PC, FEAT], F32),
    "onehot12": ([BL, PC], BF16),
    "onehot12f": ([BL, PC], F32),
    "ones128": ([1, 128], BF16),
    "objfeat": ([NROW, OBJ_D], F32),
}
_IN_LISTS = {
    "Wih_feat": (4 * H // NT_IH, [128, KF, NT_IH], BF16),
    "Whh": (4 * H // NT_IH, [128, KH, NT_IH], BF16),
    "Wk_feat": (5, [128, KH, NT_K], BF16),
    "Wk_obj": (TOPN * OBJ_D // NT_K, [128, KH, NT_K], BF16),
}
_BF16_INPUTS = {"h1T", "featT", "WfT", "Wih_emb65", "ctxT", "WcT", "WoT",
                "Wih_feat", "Whh", "Wk_feat", "Wk_obj", "onehot12", "ones128"}
_OUT_SPECS = {
    "out_h1": [BL, H],
    "out_c1": [BL, H],
    "out_htilde": [BL, H],
    "out_attn": [BL, S],
    "out_logit": [BL, C],
}
if os.environ.get("KDEBUG") == "1":
    _OUT_SPECS.update({
        "dbg_i3": [PC, TOPN],
        "dbg_simval": [PC, TOPN],
        "dbg_rowf": [PC, TOPN],
        "dbg_gsum": [128, TOPN],
    })


def compile_kernel():
    global _COMPILED
    if _COMPILED is not None:
        return _COMPILED
    nc = bacc.Bacc("TRN2", target_bir_lowering=False, debug=False,
                   num_devices=NCORES)
    ins = {}
    for name, (shape, dt) in _IN_SPECS.items():
        ins[name] = nc.dram_tensor(name, shape, dt, kind="ExternalInput").ap()
    for name, (cnt, shape, dt) in _IN_LISTS.items():
        ins[name] = [
            nc.dram_tensor(f"{name}_{j}", shape, dt, kind="ExternalInput").ap()
            for j in range(cnt)
        ]
    outs = {
        name: nc.dram_tensor(name, shape, F32, kind="ExternalOutput").ap()
        for name, shape in _OUT_SPECS.items()
    }
    with tile.TileContext(nc) as tc:
        build_kernel(tc, ins, outs)
    nc.compile()
    _COMPILED = nc
    return nc


def make_in_maps(inputs):
    in_maps = []
    for i in range(NCORES):
        m = _stage_core(i, inputs)
        flat = {}
        for k, v in m.items():
            dt16 = k in _BF16_INPUTS
            if isinstance(v, list):
                for j, arr in enumerate(v):
                    if dt16:
                        arr = arr.astype(np.float16)
                    flat[f"{k}_{j}"] = np.ascontiguousarray(arr)
            else:
                if dt16:
                    v = v.astype(np.float16)
                flat[k] = np.ascontiguousarray(v)
        in_maps.append(flat)
    return in_maps


def kernel(**inputs):
    inputs = {k: np.asarray(v) for k, v in inputs.items()}
    nc = compile_kernel()
    in_maps = make_in_maps(inputs)
    res = run_bass_kernel_spmd(nc, in_maps, core_ids=list(range(NCORES)))
    r = res.results
    h_1 = np.concatenate([r[i]["out_h1"] for i in range(NCORES)], 0)
    c_1 = np.concatenate([r[i]["out_c1"] for i in range(NCORES)], 0)
    logit = np.concatenate([r[i]["out_logit"] for i in range(NCORES)], 0)
    h_tilde = np.concatenate([r[i]["out_htilde"] for i in range(NCORES)], 0)
    ctx_attn = np.concatenate([r[i]["out_attn"] for i in range(NCORES)], 0)
    return (h_1, c_1, logit, h_tilde, ctx_attn)


if __name__ == "__main__":
    compile_kernel()
    print("compiled OK")
